# revision 1
# baseline (speedup 1.0000x reference)
"""Trainium2 Bass kernel for nn_Encoder_61753039782402 (HD-computing encoder).

Math: out[b,d] = sign( sum_f parity( sum_t L[q(b,t,f), d-t] + sum_t id[f, d-t] ) - 20.5 )
where q(b,t,f) = trunc(16*x[b,t,f] - 1) wrapped mod 16 (x==0 -> 15).

Implementation per core (D sharded 8 ways, 256 output columns each):
  - one-hot level masks OH_q[t,(b,f)] built via an exact floor trick; the 15
    equality compares are split across DVE and GPSIMD
  - shifted-L "circulant band" tiles SL_q[u,d'] = L[q, d0+d'-127+u] gathered by
    overlapping strided DMAs from the fp8 L window input (time axis reversed
    so all strides are positive; x is passed time-reversed)
  - PSUM-accumulated fp8 DoubleRow matmul chain: 8 level-pair passes + 1
    triangular-constant pass folding in the id window-sum (all operands are
    exact 0/1 in fp8e4m3; fp32 PSUM accumulation is exact)
  - parity (int convert + bitwise and) + grouped reduce over f + threshold
    to +-1; per-chunk contiguous DMA to a d-major [256, 8] output (the host
    transposes during assembly)
Host-side prep is layout/dtype only: slicing the doubled tables per core,
time-reversing/transposing x, int->fp8 casts of 0/1 tables, transposing
each core's [256, 8] output slice.
"""

from contextlib import ExitStack

import numpy as np
import ml_dtypes

import concourse.bass as bass
import concourse.bacc as bacc
import concourse.mybir as mybir
import concourse.tile as tile
from concourse.bass_utils import run_bass_kernel_spmd

B, T, F, Q, D = 8, 128, 40, 16, 2048
NCORE = 8
DS = D // NCORE  # 256 output columns per core
W = 384          # per-core window-slice width for lwb / idt
BF = B * F       # 320
f32, bf16, i32 = mybir.dt.float32, mybir.dt.bfloat16, mybir.dt.int32
f8 = mybir.dt.float8e4
AL = mybir.AluOpType
TWO23 = float(2 ** 23)

PARITY_MODE = "int"   # "mod" (single fused fp-mod op) fails walrus tensor_scalar_valid_ops
N_POOL_CMP = 7         # of the 14 plain equality compares, how many go to GPSIMD


def emit_kernel(nc, tc, ctx, xt_d, lwb_d, idt_d, out_d):
    sb = ctx.enter_context(tc.tile_pool(name="sb", bufs=1))
    psp = ctx.enter_context(tc.tile_pool(name="psp", bufs=1, space=bass.MemorySpace.PSUM))

    # ---- input DMAs ------------------------------------------------------
    xt = sb.tile([T, B, F], f32, tag="xt")
    nc.sync.dma_start(out=xt[:], in_=xt_d)
    xt2 = xt[:].rearrange("u b f -> u (b f)")  # [128, 320]

    # shifted-L gathers: sl[u, q, d'] = Lw[q, u + d']   (overlapping reads)
    # issued from ACT's HWDGE path to overlap with SP's x trigger
    sla = sb.tile([128, Q, DS], f8, tag="sla")
    for g in range(4):
        src = bass.AP(tensor=lwb_d.tensor, offset=g * 4 * W,
                      ap=[[1, 128], [W, 4], [1, DS]])
        nc.scalar.dma_start(out=sla[:, g * 4:(g + 1) * 4, :], in_=src)

    # id window slice, transposed [src, f]: one 3-chunk DMA via Pool SWDGE
    idb = sb.tile([128, 3, F], f8, tag="idb")
    nc.gpsimd.dma_start(out=idb[:], in_=idt_d.rearrange("(j p) f -> p j f", p=128))

    # ---- GPSIMD-side constants ------------------------------------------
    iot = sb.tile([128, 128], i32, tag="iot")
    nc.gpsimd.iota(out=iot[:], pattern=[[-1, 128]], base=0, channel_multiplier=1)  # p - m
    tri = sb.tile([128, 2, 128], f8, tag="tri")
    nc.gpsimd.tensor_single_scalar(out=tri[:, 0, :], in_=iot[:], scalar=0, op=AL.is_gt)  # m < p
    nc.gpsimd.tensor_single_scalar(out=tri[:, 1, :], in_=iot[:], scalar=0, op=AL.is_le)  # m >= p

    # replicate id window over b (log-doubling) on GPSIMD
    idr = sb.tile([128, 3, B, F], f8, tag="idr")
    nc.gpsimd.tensor_copy(out=idr[:, :, 0, :], in_=idb[:])
    nc.gpsimd.tensor_copy(out=idr[:, :, 1:2, :], in_=idr[:, :, 0:1, :])
    nc.gpsimd.tensor_copy(out=idr[:, :, 2:4, :], in_=idr[:, :, 0:2, :])
    nc.gpsimd.tensor_copy(out=idr[:, :, 4:8, :], in_=idr[:, :, 0:4, :])

    # ---- exact floor(16x) via round-to-nearest + fixup (DVE) ------------
    t1 = sb.tile([T, BF], f32, tag="t1")
    nc.vector.tensor_scalar(out=t1[:], in0=xt2, scalar1=16.0, scalar2=TWO23,
                            op0=AL.mult, op1=AL.add)
    t2 = sb.tile([T, BF], f32, tag="t2")
    nc.vector.tensor_single_scalar(out=t2[:], in_=t1[:], scalar=TWO23, op=AL.subtract)
    t3 = sb.tile([T, BF], f32, tag="t3")
    nc.vector.scalar_tensor_tensor(out=t3[:], in0=t2[:], scalar=0.0625, in1=xt2,
                                   op0=AL.mult, op1=AL.is_gt)
    ub = sb.tile([T, BF], bf16, tag="ub")
    nc.vector.tensor_tensor(out=ub[:], in0=t2[:], in1=t3[:], op=AL.subtract)

    # ---- one-hot level masks --------------------------------------------
    # level q <=> u == q+1 for q in 1..14; q0 <=> u<=1 minus the x==0 case;
    # q15 <=> x == 0. Plain equality masks first (they gate the matmuls),
    # zero-mask and fused q0 afterwards.
    oha = sb.tile([T, Q, BF], f8, tag="oha")
    nc.gpsimd.tensor_single_scalar(out=oha[:, Q - 1, :], in_=xt2, scalar=0.0,
                                   op=AL.is_equal)
    for q in [2, 3, 4, 5, 6, 7, 1] + list(range(8, Q - 1)):
        eng = nc.gpsimd if q >= Q - 1 - N_POOL_CMP else nc.vector
        eng.tensor_single_scalar(out=oha[:, q, :], in_=ub[:], scalar=float(q + 1),
                                 op=AL.is_equal)
    nc.vector.scalar_tensor_tensor(out=oha[:, 0, :], in0=ub[:], scalar=1.0,
                                   in1=oha[:, Q - 1, :],
                                   op0=AL.is_le, op1=AL.subtract)

    # ---- matmul chains ---------------------------------------------------
    # DoubleRow fp8 passes: two K-chunks per matmul. Pair order puts the
    # plain equality masks first, the q0/q15 pair (which needs the zero mask)
    # last, then the id-window band pair.
    pairs = [(8, 9), (2, 3), (10, 11), (4, 5), (12, 13), (6, 7), (14, 15), (0, 1)]
    DR = mybir.MatmulPerfMode.DoubleRow
    # chunk 0's whole output path (parity -> threshold -> DMA) is emitted
    # before chunk 1's matmuls so it overlaps them; only chunk 1's path is
    # kernel-tail.
    for mc in range(2):
        p = psp.tile([128, BF], f32, tag=f"acc{mc}")
        # the id-window band pass only needs iota/id tiles (ready ~2us) -> first
        j_lo = 0 if mc == 0 else 1
        nc.tensor.matmul(p[:], tri[:], idr[:, j_lo:j_lo + 2],
                         start=True, stop=False, perf_mode=DR)
        for ci, (qa, qb) in enumerate(pairs):
            assert qb == qa + 1
            nc.tensor.matmul(p[:], sla[:, qa:qb + 1, mc * 128:(mc + 1) * 128],
                             oha[:, qa:qb + 1, :],
                             start=False, stop=(ci == len(pairs) - 1), perf_mode=DR)

        si = sb.tile([128, BF], i32, tag=f"si{mc}")
        nc.vector.tensor_copy(out=si[:], in_=p[:])
        seq = sb.tile([128, BF], i32, tag=f"seq{mc}")
        nc.vector.tensor_single_scalar(out=seq[:], in_=si[:], scalar=1,
                                       op=AL.bitwise_and)
        red = sb.tile([128, B], i32, tag=f"red{mc}")
        with nc.allow_low_precision(reason="exact small-int accumulation (<=40)"):
            nc.vector.tensor_reduce(out=red[:], in_=seq[:].rearrange("p (b f) -> p b f", b=B),
                                    axis=mybir.AxisListType.X, op=AL.add)
        fin0 = sb.tile([128, B], f32, tag=f"fin0{mc}")
        nc.vector.tensor_scalar(out=fin0[:], in0=red[:], scalar1=20, scalar2=2.0,
                                op0=AL.is_gt, op1=AL.mult)
        fin = sb.tile([128, B], f32, tag=f"fin{mc}")
        nc.vector.tensor_single_scalar(out=fin[:], in_=fin0[:], scalar=1.0,
                                       op=AL.subtract)
        eng = nc.gpsimd if mc == 0 else nc.sync
        eng.dma_start(out=out_d[mc * 128:(mc + 1) * 128, :], in_=fin[:])


def build_nc():
    nc = bacc.Bacc("TRN2", target_bir_lowering=False, debug=False)
    xt_d = nc.dram_tensor("xt", [T, B, F], f32, kind="ExternalInput")
    lwb_d = nc.dram_tensor("lwb", [Q, W], f8, kind="ExternalInput")
    idt_d = nc.dram_tensor("idt", [W, F], f8, kind="ExternalInput")
    out_d = nc.dram_tensor("out", [DS, B], f32, kind="ExternalOutput")
    with tile.TileContext(nc) as tc:
        with ExitStack() as ctx:
            emit_kernel(nc, tc, ctx, xt_d[:], lwb_d[:], idt_d[:], out_d[:])
    nc.compile()
    return nc


def make_in_maps(x, level_hvs, id_hvs):
    x = np.asarray(x, dtype=np.float32)
    L = np.asarray(level_hvs, dtype=np.int32)
    ID = np.asarray(id_hvs, dtype=np.int32)
    # time-reverse + transpose to [T, B, F] (layout only)
    xt = np.ascontiguousarray(x[:, ::-1, :].transpose(1, 0, 2))
    LL2 = np.concatenate([L, L], axis=1).astype(ml_dtypes.float8_e4m3)
    II2 = np.concatenate([ID, ID], axis=1).astype(ml_dtypes.float8_e4m3)
    in_maps = []
    for c in range(NCORE):
        d0 = c * DS
        s = (d0 - 127) % D
        lwb_c = np.ascontiguousarray(LL2[:, s:s + W])
        s2 = (d0 - 128) % D
        idt_c = np.ascontiguousarray(II2[:, s2:s2 + W].T)
        in_maps.append({"xt": xt, "lwb": lwb_c, "idt": idt_c})
    return in_maps


_NC_CACHE = {}


def kernel(x, level_hvs, id_hvs):
    if "nc" not in _NC_CACHE:
        _NC_CACHE["nc"] = build_nc()
    nc = _NC_CACHE["nc"]
    in_maps = make_in_maps(x, level_hvs, id_hvs)
    res = run_bass_kernel_spmd(nc, in_maps, list(range(NCORE)))
    full = np.empty((B, D), dtype=np.float32)
    for c in range(NCORE):
        full[:, c * DS:(c + 1) * DS] = res.results[c]["out"].T
    return full



# revision 2
# speedup vs baseline: 1.1843x; 1.1843x over previous
"""Trainium2 Bass kernel for nn_Encoder_61753039782402 (HD-computing encoder).

Math: out[b,d] = sign( sum_f parity( sum_t L[q(b,t,f), d-t] + sum_t id[f, d-t] ) - 20.5 )
where q(b,t,f) = trunc(16*x[b,t,f] - 1) wrapped mod 16 (x==0 -> 15).

Telescoped step-mask formulation (removes the floor chain and one-hot
equality compares from the critical path): with step masks
s_k = [x >= k/16] (k=2..15), z = [x==0], ones = 1,

  sum_q SL_q^T OH_q = SL_0^T*ones + sum_{k=2..15} (SL_{k-1}-SL_{k-2})^T s_k
                      + (SL_15-SL_0)^T z

so the moving matmul operands are direct compares on x (f32, exact), and the
stationary operands are banded difference tables D_k in {-1,0,1} (fp8 exact),
prepared on the host from the tiny level table.

Implementation per core (D sharded 8 ways, 256 output columns each):
  - 14 step compares + z split across DVE (11) and GPSIMD (4) + ones memset
  - banded "circulant" stationary tiles gathered by 2 overlapping strided
    DMAs from the fp8 dwb window input (time axis reversed so strides are
    positive; x is passed time-reversed)
  - PSUM-accumulated fp8 DoubleRow matmul chain per 128-column chunk:
    1 triangular id-window pass + 8 level pairs, emitted in operand-readiness
    order; both chunks share one 2-bank PSUM tile
  - merged tail: one PSUM->i16 convert copy over both chunks, fused AND,
    grouped reduce over f, threshold to +-1 in fp8, single output DMA
Host-side prep: slicing/differencing the tiny tables per core,
time-reversing/transposing x, int->fp8 casts, final [128,2,8] transpose.
"""

from contextlib import ExitStack

import numpy as np
import ml_dtypes

import concourse.bass as bass
import concourse.bacc as bacc
import concourse.mybir as mybir
import concourse.tile as tile
from concourse.bass_utils import run_bass_kernel_spmd

B, T, F, Q, D = 8, 128, 40, 16, 2048
NCORE = 8
DS = D // NCORE  # 256 output columns per core
W = 384          # per-core window-slice width
BF = B * F       # 320
f32, bf16, i32, i16 = mybir.dt.float32, mybir.dt.bfloat16, mybir.dt.int32, mybir.dt.int16
f8 = mybir.dt.float8e4
AL = mybir.AluOpType

# mask slot i (0..13) holds s_{i+2}; slot 14 = ones; slot 15 = z
N_DVE_CMP = 11  # s2..s11 + z on DVE; s12..s15 on GPSIMD


def emit_kernel(nc, tc, ctx, xt_d, dwb_d, idtri_d, out_d):
    sb = ctx.enter_context(tc.tile_pool(name="sb", bufs=1))
    psp = ctx.enter_context(tc.tile_pool(name="psp", bufs=1, space=bass.MemorySpace.PSUM))

    # ---- input DMAs ------------------------------------------------------
    xt = sb.tile([T, B, F], f32, tag="xt")
    nc.sync.dma_start(out=xt[:], in_=xt_d)
    xt2 = xt[:].rearrange("u b f -> u (b f)")  # [128, 320]

    # banded stationary gathers: sla[u, r, d'] = dwb[r, u + d']
    sla = sb.tile([128, Q, DS], f8, tag="sla")
    for g, eng in ((0, nc.scalar), (1, nc.sync)):
        src = bass.AP(tensor=dwb_d.tensor, offset=g * 8 * W,
                      ap=[[1, 128], [W, 8], [1, DS]])
        eng.dma_start(out=sla[:, g * 8:(g + 1) * 8, :], in_=src)

    # tri constant [128,2,128] + replicated id window [128,3,320], one DMA
    idtri = sb.tile([128, 2 * 128 + 3 * BF], f8, tag="idtri")
    nc.gpsimd.dma_start(out=idtri[:], in_=idtri_d)
    tri = idtri[:, 0:256].rearrange("p (j m) -> p j m", j=2)
    idr = idtri[:, 256:256 + 3 * BF].rearrange("p (j e) -> p j e", j=3)

    # ---- masks -----------------------------------------------------------
    oha = sb.tile([T, Q, BF], f8, tag="oha")
    nc.gpsimd.memset(oha[:, 14, :], 1.0)
    # GPSIMD: s12..s15 (slots 10..13)
    for i in range(10, 14):
        nc.gpsimd.tensor_single_scalar(out=oha[:, i, :], in_=xt2,
                                       scalar=(i + 2) / 16.0, op=AL.is_ge)
    # DVE: s2..s11 (slots 0..9), then z (slot 15)
    for i in range(10):
        nc.vector.tensor_single_scalar(out=oha[:, i, :], in_=xt2,
                                       scalar=(i + 2) / 16.0, op=AL.is_ge)
    nc.vector.tensor_single_scalar(out=oha[:, 15, :], in_=xt2,
                                   scalar=0.0, op=AL.is_equal)

    # ---- matmul chains ---------------------------------------------------
    # per chunk: tri/id pass + 8 DoubleRow fp8 pair passes, in readiness order
    DR = mybir.MatmulPerfMode.DoubleRow
    pacc = psp.tile([128, 1024], f32, tag="pacc")  # 2 banks; chunk mc at [:, mc*512:+320]
    pair_order = [0, 1, 2, 5, 3, 6, 4, 7]  # by (mask, dwb-DMA) readiness
    for mc in range(2):
        nc.tensor.matmul(pacc[:, mc * 512:mc * 512 + BF], tri, idr[:, mc:mc + 2],
                         start=True, stop=False, perf_mode=DR)
    for ci, pi in enumerate(pair_order):
        for mc in range(2):
            nc.tensor.matmul(pacc[:, mc * 512:mc * 512 + BF],
                             sla[:, 2 * pi:2 * pi + 2, mc * 128:(mc + 1) * 128],
                             oha[:, 2 * pi:2 * pi + 2, :],
                             start=False, stop=(ci == len(pair_order) - 1),
                             perf_mode=DR)

    # ---- merged tail: parity -> grouped reduce -> threshold --------------
    pv = pacc[:].rearrange("p (c k) -> p c k", c=2)[:, :, 0:BF]  # [128, 2, 320]
    si = sb.tile([128, 2, BF], i16, tag="si")
    nc.vector.tensor_copy(out=si[:], in_=pv)
    seq = sb.tile([128, 2 * BF], i16, tag="seq")
    nc.vector.tensor_single_scalar(out=seq[:], in_=si[:].rearrange("p c k -> p (c k)"),
                                   scalar=1, op=AL.bitwise_and)
    red = sb.tile([128, 2 * B], i16, tag="red")
    with nc.allow_low_precision(reason="exact small-int accumulation (<=40)"):
        nc.vector.tensor_reduce(out=red[:], in_=seq[:].rearrange("p (g f) -> p g f", f=F),
                                axis=mybir.AxisListType.X, op=AL.add)
    fin0 = sb.tile([128, 2 * B], f8, tag="fin0")
    nc.vector.tensor_scalar(out=fin0[:], in0=red[:], scalar1=20, scalar2=2.0,
                            op0=AL.is_gt, op1=AL.mult)
    fin = sb.tile([128, 2 * B], f8, tag="fin")
    nc.vector.tensor_single_scalar(out=fin[:], in_=fin0[:], scalar=1.0,
                                   op=AL.subtract)
    nc.sync.dma_start(out=out_d, in_=fin[:])


def build_nc():
    nc = bacc.Bacc("TRN2", target_bir_lowering=False, debug=False)
    xt_d = nc.dram_tensor("xt", [T, B, F], f32, kind="ExternalInput")
    dwb_d = nc.dram_tensor("dwb", [Q, W], f8, kind="ExternalInput")
    idtri_d = nc.dram_tensor("idtri", [128, 2 * 128 + 3 * BF], f8, kind="ExternalInput")
    out_d = nc.dram_tensor("out", [128, 2 * B], f8, kind="ExternalOutput")
    with tile.TileContext(nc) as tc:
        with ExitStack() as ctx:
            emit_kernel(nc, tc, ctx, xt_d[:], dwb_d[:], idtri_d[:], out_d[:])
    nc.compile()
    return nc


def make_in_maps(x, level_hvs, id_hvs):
    x = np.asarray(x, dtype=np.float32)
    L = np.asarray(level_hvs, dtype=np.int32)
    ID = np.asarray(id_hvs, dtype=np.int32)
    # time-reverse + transpose to [T, B, F] (layout only)
    xt = np.ascontiguousarray(x[:, ::-1, :].transpose(1, 0, 2))
    L2 = np.concatenate([L, L], axis=1)
    II2 = np.concatenate([ID, ID], axis=1)
    # constant triangular masks for the id-window pass
    p_ = np.arange(128)[:, None]
    m_ = np.arange(128)[None, :]
    tri = np.empty((128, 2, 128), dtype=np.int32)
    tri[:, 0, :] = p_ > m_
    tri[:, 1, :] = p_ <= m_
    in_maps = []
    for c in range(NCORE):
        d0 = c * DS
        s0 = (d0 - 127) % D
        Lw = L2[:, s0:s0 + W]
        # dwb rows: 0..13 = D_k = SL_{k-1}-SL_{k-2} (k=i+2); 14 = SL_0; 15 = E
        dwb = np.empty((Q, W), dtype=np.int32)
        dwb[0:14] = Lw[1:15] - Lw[0:14]
        dwb[14] = Lw[0]
        dwb[15] = Lw[15] - Lw[0]
        s2 = (d0 - 128) % D
        win = II2[:, s2:s2 + W]                                  # [F, 384]
        A = win.T.reshape(3, 128, F).transpose(1, 0, 2)          # [p, j, f]
        idr = np.broadcast_to(A[:, :, None, :], (128, 3, B, F))
        idtri = np.concatenate(
            [tri.reshape(128, 256), idr.reshape(128, 3 * BF)], axis=1)
        in_maps.append({
            "xt": xt,
            "dwb": dwb.astype(ml_dtypes.float8_e4m3),
            "idtri": idtri.astype(ml_dtypes.float8_e4m3),
        })
    return in_maps


_NC_CACHE = {}


def kernel(x, level_hvs, id_hvs):
    if "nc" not in _NC_CACHE:
        _NC_CACHE["nc"] = build_nc()
    nc = _NC_CACHE["nc"]
    in_maps = make_in_maps(x, level_hvs, id_hvs)
    res = run_bass_kernel_spmd(nc, in_maps, list(range(NCORE)))
    full = np.empty((B, D), dtype=np.float32)
    for c in range(NCORE):
        r = res.results[c]["out"].astype(np.float32).reshape(128, 2, B)
        full[:, c * DS:(c + 1) * DS] = r.transpose(1, 0, 2).reshape(DS, B).T
    return full


# revision 27
# speedup vs baseline: 1.5095x; 1.2746x over previous
"""Trainium2 Bass kernel for nn_Encoder_61753039782402 (HD-computing encoder).

Math: out[b,d] = sign( sum_f parity( sum_t L[q(b,t,f), d-t] + sum_t id[f, d-t] ) - 20.5 )
where q(b,t,f) = trunc(16*x[b,t,f] - 1) wrapped mod 16 (x==0 -> 15).

Telescoped step-mask formulation: with s_k = [x >= k/16] (k=2..15),

  sum_q SL_q^T OH_q = SL_0^T*ones + sum_{k=2..15} (SL_{k-1}-SL_{k-2})^T s_k
                      + (SL_15-SL_0)^T [x==0]

The moving matmul operands become direct compares on x (exact, no floor
chain); the stationary operands are banded difference tables D_k in {-1,0,1}
(fp8 exact).  Three masks are computed on the ACT engine as sign(x - k/16)
in {-1,+1}; their D rows are halved on the host (still fp8-exact) and the
resulting constant offset joins the SL_0^T*ones term, which is folded into
the parity step as a per-partition scalar.  The [x==0] term and an
ACT-free mask path are only used in a lazily compiled safe variant when x
contains exact zeros or exact k/16 boundary values (never for continuous
uniform inputs).

Per core (D sharded 8 ways, 256 output columns each):
  - 14 step masks split DVE (8 compares) / GPSIMD (3) / ACT (3 signs),
    started as soon as x lands
  - D_k pair rows byte-interleaved on the host so the banded overlapping
    gather DMA reads 512B-contiguous runs (full DMA bus rate); pair rows
    host-permuted so the first gather DMA carries the pairs whose masks are
    ready earliest
  - warm-up matmuls on a scratch PSUM bank ramp the PE p-state to full
  - per chunk: 1 triangular id-window pass + 7 fp8 DoubleRow D-pairs in
    operand-readiness order; separate PSUM tiles per chunk so the two
    tail readers (DVE / ACT) stay independent
  - tail: per-chunk PSUM->i16 convert (+const) on DVE / ACT, parity AND on
    DVE / GPSIMD, per-chunk grouped reduce over f on DVE, single ACT
    sign(red - 20.5) threshold to +-1 in fp8, single output DMA
"""

from contextlib import ExitStack

import numpy as np
import ml_dtypes

import concourse.bass as bass
import concourse.bacc as bacc
import concourse.mybir as mybir
import concourse.tile as tile
from concourse.bass_utils import run_bass_kernel_spmd

B, T, F, Q, D = 8, 128, 40, 16, 2048
NCORE = 8
DS = D // NCORE  # 256 output columns per core
W = 384          # per-core window-slice width
BF = B * F       # 320
f32, i16 = mybir.dt.float32, mybir.dt.int16
f8 = mybir.dt.float8e4
AL = mybir.AluOpType
AF = mybir.ActivationFunctionType

N_WARM = 5       # PE p-state warm-up matmuls
WARM_N = 512     # warm-up moving free size

# mask slot i (0..13) holds s_{i+2}; engine split (fast variant):
DVE_SLOTS = list(range(0, 8))      # s2..s9   (0/1 compares)
ACT_SLOTS = [8, 9, 13]             # s10, s11, s15 as sign(x-k/16) in {-1,+1}
POOL_SLOTS = [10, 11, 12]          # s12..s14 (0/1 compares)
# pair pi uses mask slots (2pi, 2pi+1); dwb row-pair position in the gather
# is permuted so the first DMA group has the earliest-ready pairs
PAIR_POS = {0: 0, 1: 1, 4: 2, 5: 3, 2: 4, 6: 5, 3: 6}
CHAIN = [0, 1, 4, 5, 2, 6, 3]      # emission order; last pair stops the chain


def emit_kernel(nc, tc, ctx, xt_d, dwb_d, idtri_d, out_d, safe):
    sb = ctx.enter_context(tc.tile_pool(name="sb", bufs=1))
    psp = ctx.enter_context(tc.tile_pool(name="psp", bufs=1, space=bass.MemorySpace.PSUM))
    npair = 8 if safe else 7

    # ---- input DMAs ------------------------------------------------------
    xt = sb.tile([T, B, F], f32, tag="xt")
    nc.sync.dma_start(out=xt[:], in_=xt_d)
    xt2 = xt[:].rearrange("u b f -> u (b f)")  # [128, 320]

    # banded stationary gathers, pair-interleaved: sla[u, i, v] = dwbp[i][2u+v]
    # (512B contiguous runs -> full DMA bus rate)
    sla = sb.tile([128, npair, 2 * DS], f8, tag="sla")
    groups = (((0, 2, nc.scalar), (2, 3, nc.sync), (5, npair - 5, nc.sync))
              if not safe else ((0, 4, nc.scalar), (4, npair - 4, nc.sync)))
    for (ga, gn, eng) in groups:
        src = bass.AP(tensor=dwb_d.tensor, offset=ga * 2 * W + 2,
                      ap=[[2, 128], [2 * W, gn], [1, 2 * DS]])
        eng.dma_start(out=sla[:, ga:ga + gn, :], in_=src)

    # tri constant [128,2,128] + replicated id window [128,3,320] + the
    # per-partition f32 constants (SL0 window sums + sign-mask offset,
    # bitcast-packed as 8 trailing bytes), one DMA
    idtri = sb.tile([128, 2 * 128 + 3 * BF + 8], f8, tag="idtri")
    nc.gpsimd.dma_start(out=idtri[:], in_=idtri_d)
    tri = idtri[:, 0:256].rearrange("p (j m) -> p j m", j=2)
    idr = idtri[:, 256:256 + 3 * BF].rearrange("p (j e) -> p j e", j=3)
    cvec = idtri[:, 256 + 3 * BF:256 + 3 * BF + 8].bitcast(f32)  # [128, 2]

    # small bias constants for ACT (bias must be a per-partition AP)
    biasv = sb.tile([T, 4], f32, tag="biasv")
    for n, i in enumerate(ACT_SLOTS):
        nc.gpsimd.memset(biasv[:, n:n + 1], -(i + 2) / 16.0)
    nc.gpsimd.memset(biasv[:, 3:4], -20.5)

    # ---- PE p-state warm-up ---------------------------------------------
    dmy = sb.tile([128, WARM_N], f8, tag="dmy")
    nc.vector.memset(dmy[:], 0.0)
    pdmy = psp.tile([128, WARM_N], f32, tag="pdmy")
    for _ in range(N_WARM):
        nc.tensor.matmul(pdmy[:], dmy[:, 0:128], dmy[:],
                         start=True, stop=True)

    # ---- masks: step masks s_{i+2} in slot i -----------------------------
    oha = sb.tile([T, 2 * npair, BF], f8, tag="oha")
    if safe:
        nc.gpsimd.memset(oha[:, 15, :], 0.0)
        nc.vector.tensor_single_scalar(out=oha[:, 14, :], in_=xt2,
                                       scalar=0.0, op=AL.is_equal)
        for i in range(10, 14):
            nc.gpsimd.tensor_single_scalar(out=oha[:, i, :], in_=xt2,
                                           scalar=(i + 2) / 16.0, op=AL.is_ge)
        for i in range(10):
            nc.vector.tensor_single_scalar(out=oha[:, i, :], in_=xt2,
                                           scalar=(i + 2) / 16.0, op=AL.is_ge)
    else:
        for n, i in enumerate(ACT_SLOTS):
            # sign-mask in {-1,+1}; exact since x != k/16 (checked host-side)
            nc.scalar.activation(out=oha[:, i, :], in_=xt2, func=AF.Sign,
                                 bias=biasv[:, n:n + 1])
        for i in POOL_SLOTS:
            nc.gpsimd.tensor_single_scalar(out=oha[:, i, :], in_=xt2,
                                           scalar=(i + 2) / 16.0, op=AL.is_ge)
        for i in DVE_SLOTS:
            nc.vector.tensor_single_scalar(out=oha[:, i, :], in_=xt2,
                                           scalar=(i + 2) / 16.0, op=AL.is_ge)

    # ---- matmul chains ---------------------------------------------------
    DR = mybir.MatmulPerfMode.DoubleRow
    DRI = mybir.MatmulPerfMode.DoubleRowSwInterleave
    # separate PSUM tiles per chunk: readers on different engines stay independent
    pacc0 = psp.tile([128, 512], f32, tag="pacc0")
    pacc1 = psp.tile([128, 512], f32, tag="pacc1")
    pacc = [pacc0, pacc1]
    chain = CHAIN + ([7] if safe else [])
    pair_pos = dict(PAIR_POS, **({7: 7} if safe else {}))
    for mc in range(2):
        nc.tensor.matmul(pacc[mc][:, 0:BF], tri, idr[:, mc:mc + 2],
                         start=True, stop=False, perf_mode=DR)
    for ci, pi in enumerate(chain):
        for mc in range(2):
            pos = pair_pos[pi]
            st = sla[:, pos, (1 - mc) * 256:(1 - mc) * 256 + 256]
            nc.tensor.matmul(pacc[mc][:, 0:BF],
                             st, oha[:, 2 * pi:2 * pi + 2, :],
                             start=False, stop=(ci == len(chain) - 1),
                             perf_mode=DRI)

    # ---- tail: (+c, parity) -> grouped reduce -> threshold ---------------
    # separate per-chunk tiles so DVE/ACT/Pool stages run without false deps
    si0 = sb.tile([128, BF], i16, tag="si0")
    si1 = sb.tile([128, BF], i16, tag="si1")
    nc.vector.tensor_single_scalar(out=si0[:], in_=pacc0[:, 0:BF],
                                   scalar=cvec[:, 0:1], op=AL.add)
    nc.scalar.activation(out=si1[:], in_=pacc1[:, 0:BF], func=AF.Identity,
                         bias=cvec[:, 1:2])
    seq = sb.tile([128, 2, BF], i16, tag="seq")
    nc.vector.tensor_single_scalar(out=seq[:, 0, :], in_=si0[:],
                                   scalar=1, op=AL.bitwise_and)
    nc.vector.tensor_single_scalar(out=seq[:, 1, :], in_=si1[:],
                                   scalar=1, op=AL.bitwise_and)
    red = sb.tile([128, 2 * B], i16, tag="red")
    with nc.allow_low_precision(reason="exact small-int accumulation (<=40)"):
        nc.vector.tensor_reduce(out=red[:],
                                in_=seq[:].rearrange("p c (b f) -> p (c b) f", f=F),
                                axis=mybir.AxisListType.X, op=AL.add)
    fin = sb.tile([128, 2 * B], f8, tag="fin")
    fin0 = sb.tile([128, 2 * B], f8, tag="fin0")
    nc.vector.tensor_scalar(out=fin0[:], in0=red[:], scalar1=20, scalar2=2.0,
                            op0=AL.is_gt, op1=AL.mult)
    nc.vector.tensor_single_scalar(out=fin[:], in_=fin0[:], scalar=1.0,
                                   op=AL.subtract)
    nc.sync.dma_start(out=out_d, in_=fin[:])


def build_nc(safe):
    npair = 8 if safe else 7
    nc = bacc.Bacc("TRN2", target_bir_lowering=False, debug=False)
    xt_d = nc.dram_tensor("xt", [T, B, F], f32, kind="ExternalInput")
    dwb_d = nc.dram_tensor("dwb", [npair, 2 * W], f8, kind="ExternalInput")
    idtri_d = nc.dram_tensor("idtri", [128, 2 * 128 + 3 * BF + 8], f8, kind="ExternalInput")
    out_d = nc.dram_tensor("out", [128, 2 * B], f8, kind="ExternalOutput")
    with tile.TileContext(nc) as tc:
        with ExitStack() as ctx:
            emit_kernel(nc, tc, ctx, xt_d[:], dwb_d[:], idtri_d[:],
                        out_d[:], safe)
    nc.compile()
    return nc


def make_in_maps(x, level_hvs, id_hvs, safe):
    x = np.asarray(x, dtype=np.float32)
    L = np.asarray(level_hvs, dtype=np.int32)
    ID = np.asarray(id_hvs, dtype=np.int32)
    npair = 8 if safe else 7
    # transpose to [T, B, F] (layout only; time axis NOT reversed -- the
    # SwInterleave column reversal supplies the band direction)
    xt = np.ascontiguousarray(x.transpose(1, 0, 2))
    L2 = np.concatenate([L, L], axis=1)
    II2 = np.concatenate([ID, ID], axis=1)
    p_ = np.arange(128)[:, None]
    m_ = np.arange(128)[None, :]
    tri = np.empty((128, 2, 128), dtype=np.int32)
    tri[:, 0, :] = p_ > m_
    tri[:, 1, :] = p_ <= m_
    act_set = set() if safe else set(ACT_SLOTS)
    in_maps = []
    for c in range(NCORE):
        d0 = c * DS
        s0 = (d0 - 127) % D
        Lw = L2[:, s0:s0 + W].astype(np.float64)
        # D_k = SL_{k-1} - SL_{k-2} (k=2..15); ACT sign-mask rows halved
        Dk = Lw[1:15] - Lw[0:14]                              # [14, 384]
        half = np.ones((14, 1))
        for i in act_set:
            half[i] = 0.5
        Dh = Dk * half
        # rows column-REVERSED then pair-interleaved bytewise (the
        # DoubleRowSwInterleave weight convention), pairs host-permuted
        dwbp = np.zeros((npair, 2 * W), dtype=np.float64)
        for pi in range(7):
            pos = PAIR_POS[pi] if not safe else dict(PAIR_POS, **{7: 7})[pi]
            pr = Dh[2 * pi:2 * pi + 2, ::-1]                  # [2, 384] reversed
            dwbp[pos] = pr.T.reshape(2 * W)
        if safe:  # pair 7 = (E, 0) matched with masks ([x==0], 0)
            dwbp[7, 0::2] = (Lw[15] - Lw[0])[::-1]
        # per-partition constants: SL0 window sums + half-sum of sign rows
        base = Lw[0] + 0.5 * Dk[list(act_set)].sum(axis=0) if act_set else Lw[0]
        cs = np.concatenate([[0.0], np.cumsum(base)])
        winsum = cs[128:128 + 256] - cs[0:256]
        cvec = np.ascontiguousarray(
            winsum.reshape(2, 128).T.astype(np.float32))      # [128, 2]
        # id window, replicated over b, plus tri constant
        s2 = (d0 - 128) % D
        win = II2[:, s2:s2 + W]                               # [F, 384]
        A = win.T.reshape(3, 128, F).transpose(1, 0, 2)       # [p, j, f]
        idr = np.broadcast_to(A[:, :, None, :], (128, 3, B, F))
        idtri = np.concatenate(
            [tri.reshape(128, 256).astype(ml_dtypes.float8_e4m3).view(np.uint8),
             idr.reshape(128, 3 * BF).astype(ml_dtypes.float8_e4m3).view(np.uint8),
             cvec.view(np.uint8)], axis=1)
        in_maps.append({
            "xt": xt,
            "dwb": dwbp.astype(ml_dtypes.float8_e4m3),
            "idtri": np.ascontiguousarray(idtri).view(ml_dtypes.float8_e4m3),
        })
    return in_maps


_NC_CACHE = {}


def kernel(x, level_hvs, id_hvs):
    xa = np.asarray(x, dtype=np.float32)
    # safe variant when x has exact zeros (wrap-to-level-15 term needed) or
    # exact bucket-boundary values (ACT sign-mask would misclassify)
    safe = bool((xa == 0).any())
    if not safe:
        for i in ACT_SLOTS:
            if (xa == np.float32((i + 2) / 16.0)).any():
                safe = True
                break
    key = "nc_safe" if safe else "nc"
    if key not in _NC_CACHE:
        _NC_CACHE[key] = build_nc(safe)
    nc = _NC_CACHE[key]
    in_maps = make_in_maps(x, level_hvs, id_hvs, safe)
    res = run_bass_kernel_spmd(nc, in_maps, list(range(NCORE)))
    full = np.empty((B, D), dtype=np.float32)
    for c in range(NCORE):
        r = res.results[c]["out"].astype(np.float32).reshape(128, 2, B)
        full[:, c * DS:(c + 1) * DS] = r.transpose(1, 0, 2).reshape(DS, B).T
    return full
